# revision 1
# baseline (speedup 1.0000x reference)
"""Causal self-attention (B=2, T=2048, C=1024, H=16) on 8 TRN2 NeuronCores.

Sharding: core c = (batch b=c//4, head-group g=c%4); each core owns 4 heads
of one batch element. Per core:
  Q^T/K^T = W_slice @ x^T (+bias)          [feature-major layouts]
  V       = x @ Wv_slice^T                 [token-major, augmented with ones]
  S^T     = K^T_blk.T @ Q^T  (keys on partitions, queries on free dim)
  P^T     = exp(S^T / 32) * causal_mask
  [O^T; s]= [V | 1].T @ P^T  (PSUM-accumulated over key blocks)
  O_n^T   = O^T * 1/s
  y_part  = O_n^T.T @ Wp_slice^T           [proj partial over 256 of 1024 dims]
Host: y[b] = sum_g y_part[b,g] + (b_proj + W_proj @ b_attn[2C:3C]).
All matmuls run in float32r (~2e-4 rel err, 4x faster than fp32).
"""
import sys

sys.path.insert(0, "/opt/trn_rl_repo")

import numpy as np

import concourse.bass as bass  # noqa: F401
import concourse.mybir as mybir
import concourse.tile as tile
from concourse import bacc
from concourse.bass_utils import run_bass_kernel_spmd

B, T, C, H = 2, 2048, 1024, 16
HG = 4            # heads per core
D = C // H        # 64
F = HG * D        # 256 features per core for each of q/k/v
CC = C // 128     # 8 contraction chunks
NKB = T // 128    # 16 key blocks
NTB = T // 128    # 16 token blocks
NQS = T // 512    # 4 query spans
SCALE = 1.0 / 32.0
AF = mybir.ActivationFunctionType

_NC_CACHE = {}


def build_program():
    if "nc" in _NC_CACHE:
        return _NC_CACHE["nc"]
    f32r = mybir.dt.float32r
    f32 = mybir.dt.float32
    nc = bacc.Bacc("TRN2", target_bir_lowering=False, debug=False, num_devices=8)

    xT_d = nc.dram_tensor("xT", [C, T], f32r, kind="ExternalInput").ap()
    wqT_d = nc.dram_tensor("wqT", [C, F], f32r, kind="ExternalInput").ap()
    wkT_d = nc.dram_tensor("wkT", [C, F], f32r, kind="ExternalInput").ap()
    wvT_d = nc.dram_tensor("wvT", [C, F], f32r, kind="ExternalInput").ap()
    wpT_d = nc.dram_tensor("wpT", [F, C], f32r, kind="ExternalInput").ap()
    bq_d = nc.dram_tensor("bq2", [128, 2], f32, kind="ExternalInput").ap()
    bk_d = nc.dram_tensor("bk2", [128, 2], f32, kind="ExternalInput").ap()
    cm_d = nc.dram_tensor("cmask", [128, 128], f32r, kind="ExternalInput").ap()
    y_d = nc.dram_tensor("y_part", [T, C], f32, kind="ExternalOutput").ap()

    with tile.TileContext(nc) as tc:
        with tc.tile_pool(name="persist", bufs=1) as pp, \
             tc.tile_pool(name="ps_s", bufs=2, space="PSUM") as ps_s, \
             tc.tile_pool(name="ps_acc", bufs=4, space="PSUM") as ps_acc:
            # ---- persistent SBUF ----
            wq = pp.tile([128, CC, F], f32r, tag="wq")
            wk = pp.tile([128, CC, F], f32r, tag="wk")
            wv = pp.tile([128, CC, F], f32r, tag="wv")
            wp = pp.tile([128, 2, C], f32r, tag="wp")
            bq = pp.tile([128, 2], f32, tag="bq")
            bk = pp.tile([128, 2], f32, tag="bk")
            cm = pp.tile([128, 128], f32r, tag="cm")
            qt = pp.tile([128, 2, T], f32r, tag="qt")   # q features 0..255 on (part, fb)
            kt = pp.tile([128, 2, T], f32r, tag="kt")
            va = pp.tile([128, NKB, HG, 128], f32r, tag="va")  # [V | ones] per (kb, h)
            on = pp.tile([128, 2, T], f32r, tag="on")   # normalized O^T, proj lhsT

            nc.sync.dma_start(wq[:], wqT_d.rearrange("(cc p) f -> p cc f", p=128))
            nc.sync.dma_start(wk[:], wkT_d.rearrange("(cc p) f -> p cc f", p=128))
            nc.sync.dma_start(wv[:], wvT_d.rearrange("(cc p) f -> p cc f", p=128))
            nc.sync.dma_start(wp[:], wpT_d.rearrange("(ic p) o -> p ic o", p=128))
            nc.sync.dma_start(bq[:], bq_d)
            nc.sync.dma_start(bk[:], bk_d)
            nc.sync.dma_start(cm[:], cm_d)
            nc.vector.memset(va[:, :, :, D:128].bitcast(mybir.dt.float32), 1.0)

            # ---- phase 1: QKV projections (x^T resident only here) ----
            with tc.tile_pool(name="xtp", bufs=1) as xtp:
                xt = xtp.tile([128, CC, T], f32r, tag="xt")
                for cc in range(CC):
                    nc.sync.dma_start(xt[:, cc, :], xT_d[128 * cc:128 * (cc + 1), :])

                # V first so attention can start as early as possible
                for tb in range(NTB):
                    v_ps = ps_s.tile([128, F], mybir.dt.float32, tag="s",
                                     name=f"v_ps{tb}")
                    for cc in range(CC):
                        nc.tensor.matmul(v_ps[:], xt[:, cc, 128 * tb:128 * (tb + 1)],
                                         wv[:, cc, :], start=(cc == 0),
                                         stop=(cc == CC - 1))
                    nc.scalar.copy(va[:, tb, :, 0:D],
                                   v_ps[:].rearrange("p (h d) -> p h d", h=HG))

                for fb in range(2):   # feature blocks (head pairs)
                    for name, w_t, b_t, dst in (("k", wk, bk, kt), ("q", wq, bq, qt)):
                        for ts in range(NQS):
                            g_ps = ps_s.tile([128, 512], mybir.dt.float32, tag="s",
                                             name=f"{name}_ps{fb}_{ts}")
                            for cc in range(CC):
                                nc.tensor.matmul(
                                    g_ps[:],
                                    w_t[:, cc, 128 * fb:128 * (fb + 1)],
                                    xt[:, cc, 512 * ts:512 * (ts + 1)],
                                    start=(cc == 0), stop=(cc == CC - 1))
                            nc.scalar.activation(dst[:, fb, 512 * ts:512 * (ts + 1)],
                                                 g_ps[:], AF.Identity,
                                                 bias=b_t[:, fb:fb + 1], scale=1.0)

            # ---- phase 2: attention, head by head ----
            with tc.tile_pool(name="attn", bufs=1) as ap, \
                 tc.tile_pool(name="ptp", bufs=3) as ptp, \
                 tc.tile_pool(name="recp", bufs=2) as recp:
                for h in range(HG):
                    fb, po = h // 2, 64 * (h % 2)
                    oacc = [ps_acc.tile([128, 512], mybir.dt.float32, tag="acc",
                                        name=f"oacc{h}_{qi}") for qi in range(NQS)]
                    kb_last = [min(NKB - 1, 4 * qi + 3) for qi in range(NQS)]
                    for kb in range(NKB):
                        qlo = 128 * kb
                        for ch in range(qlo, T, 1024):
                            clen = min(1024, T - ch)
                            s_ps = ps_s.tile([128, clen], mybir.dt.float32, tag="s",
                                             name=f"s{h}_{kb}_{ch}")
                            for ns in range(0, clen, 512):
                                n = min(512, clen - ns)
                                nc.tensor.matmul(
                                    s_ps[:, ns:ns + n],
                                    kt[po:po + 64, fb, qlo:qlo + 128],
                                    qt[po:po + 64, fb, ch + ns:ch + ns + n],
                                    start=True, stop=True)
                            pt = ptp.tile([128, clen], f32r, tag="pt",
                                          name=f"pt{h}_{kb}_{ch}")
                            nc.scalar.activation(pt[:], s_ps[:], AF.Exp,
                                                 bias=0.0, scale=SCALE)
                            if ch == qlo:  # causal mask on the diagonal block
                                nc.vector.tensor_mul(pt[:, 0:128], pt[:, 0:128], cm[:])
                            qi0, qi1 = ch // 512, (ch + clen + 511) // 512
                            for qi in range(qi0, qi1):
                                glo, ghi = max(ch, 512 * qi), min(ch + clen, 512 * (qi + 1))
                                nc.tensor.matmul(
                                    oacc[qi][:, glo - 512 * qi:ghi - 512 * qi],
                                    va[:, kb, h, :],
                                    pt[:, glo - ch:ghi - ch],
                                    start=(kb == 0), stop=(kb == kb_last[qi]))
                    for qi in range(NQS):
                        rec = recp.tile([64, 512], mybir.dt.float32, tag="rec",
                                        name=f"rec{h}_{qi}")
                        nc.vector.reciprocal(rec[:], oacc[qi][64:128, :])
                        nc.vector.tensor_mul(on[po:po + 64, fb, 512 * qi:512 * (qi + 1)],
                                             oacc[qi][0:64, :], rec[:])

                # ---- phase 3: output projection partial ----
                with tc.tile_pool(name="outp", bufs=3) as outp:
                    for tb in range(NTB):
                        osb = outp.tile([128, C], mybir.dt.float32, tag="osb",
                                        name=f"osb{tb}")
                        for os_ in range(2):
                            p_ps = ps_s.tile([128, 512], mybir.dt.float32, tag="s",
                                             name=f"p_ps{tb}_{os_}")
                            for ic in range(2):
                                nc.tensor.matmul(p_ps[:],
                                                 on[:, ic, 128 * tb:128 * (tb + 1)],
                                                 wp[:, ic, 512 * os_:512 * (os_ + 1)],
                                                 start=(ic == 0), stop=(ic == 1))
                            nc.vector.tensor_copy(osb[:, 512 * os_:512 * (os_ + 1)],
                                                  p_ps[:])
                        nc.sync.dma_start(y_d[128 * tb:128 * (tb + 1), :], osb[:])
    nc.compile()
    _NC_CACHE["nc"] = nc
    return nc


def _in_maps(x, w_attn, b_attn, w_proj):
    cmask = (np.arange(128)[None, :] >= np.arange(128)[:, None]).astype(np.float32)
    maps = []
    for c in range(8):
        b, g = c // 4, c % 4
        r0 = F * g
        maps.append({
            "xT": np.ascontiguousarray(x[b].T),
            "wqT": np.ascontiguousarray(w_attn[r0:r0 + F, :].T),
            "wkT": np.ascontiguousarray(w_attn[C + r0:C + r0 + F, :].T),
            "wvT": np.ascontiguousarray(w_attn[2 * C + r0:2 * C + r0 + F, :].T),
            "wpT": np.ascontiguousarray(w_proj[:, r0:r0 + F].T),
            "bq2": np.ascontiguousarray(b_attn[r0:r0 + F].reshape(2, 128).T),
            "bk2": np.ascontiguousarray(b_attn[C + r0:C + r0 + F].reshape(2, 128).T),
            "cmask": cmask,
        })
    return maps


def kernel(x, mask, W_attn, b_attn, W_proj, b_proj, _trace=False, _trace_kwargs=None):
    x = np.asarray(x, dtype=np.float32)
    W_attn = np.asarray(W_attn, dtype=np.float32)
    b_attn = np.asarray(b_attn, dtype=np.float32)
    W_proj = np.asarray(W_proj, dtype=np.float32)
    b_proj = np.asarray(b_proj, dtype=np.float32)

    nc = build_program()
    maps = _in_maps(x, W_attn, b_attn, W_proj)
    res = run_bass_kernel_spmd(nc, maps, core_ids=list(range(8)),
                               trace=_trace, **(_trace_kwargs or {}))

    b_eff = b_proj + W_proj @ b_attn[2 * C:3 * C]
    y = np.empty((B, T, C), dtype=np.float32)
    for b in range(B):
        acc = np.zeros((T, C), dtype=np.float32)
        for g in range(4):
            acc += res.results[4 * b + g]["y_part"]
        y[b] = acc + b_eff
    kernel.last_results = res
    return y


# revision 5
# speedup vs baseline: 1.0680x; 1.0680x over previous
"""Causal self-attention (B=2, T=2048, C=1024, H=16) on 8 TRN2 NeuronCores.

Sharding: core c = (batch b=c//4, head-group g=c%4); each core owns 4 heads
of one batch element. Per core:
  Q^T/K^T = W_slice @ x^T (+bias)          [feature-major layouts]
  V       = x @ Wv_slice^T                 [token-major, augmented with ones]
  S^T     = K^T_blk.T @ Q^T  (keys on partitions, queries on free dim)
  P^T     = exp(S^T / 32) * causal_mask
  [O^T; s]= [V | 1].T @ P^T  (PSUM-accumulated over key blocks)
  O_n^T   = O^T * 1/s
  y_part  = O_n^T.T @ Wp_slice^T           [proj partial over 256 of 1024 dims]
Host: y[b] = sum_g y_part[b,g] + (b_proj + W_proj @ b_attn[2C:3C]).
All matmuls run in float32r (~2e-4 rel err, 4x faster than fp32).
"""
import sys

sys.path.insert(0, "/opt/trn_rl_repo")

import numpy as np

import concourse.bass as bass  # noqa: F401
import concourse.mybir as mybir
import concourse.tile as tile
from concourse import bacc
from concourse.bass_utils import run_bass_kernel_spmd

B, T, C, H = 2, 2048, 1024, 16
HG = 4            # heads per core
D = C // H        # 64
F = HG * D        # 256 features per core for each of q/k/v
CC = C // 128     # 8 contraction chunks
NKB = T // 128    # 16 key blocks
NTB = T // 128    # 16 token blocks
NQS = T // 512    # 4 query spans
SCALE = 1.0 / 32.0
AF = mybir.ActivationFunctionType

_NC_CACHE = {}


def build_program():
    if "nc" in _NC_CACHE:
        return _NC_CACHE["nc"]
    f32r = mybir.dt.bfloat16  # matmul operand dtype (fp32 PSUM accum)
    f32 = mybir.dt.float32
    nc = bacc.Bacc("TRN2", target_bir_lowering=False, debug=False, num_devices=8)

    xT_d = nc.dram_tensor("xT", [C, T], f32r, kind="ExternalInput").ap()
    wqT_d = nc.dram_tensor("wqT", [C, F], f32r, kind="ExternalInput").ap()
    wkT_d = nc.dram_tensor("wkT", [C, F], f32r, kind="ExternalInput").ap()
    wvT_d = nc.dram_tensor("wvT", [C, F], f32r, kind="ExternalInput").ap()
    wpT_d = nc.dram_tensor("wpT", [F, C], f32r, kind="ExternalInput").ap()
    bq_d = nc.dram_tensor("bq2", [128, 2], f32, kind="ExternalInput").ap()
    bk_d = nc.dram_tensor("bk2", [128, 2], f32, kind="ExternalInput").ap()
    cm_d = nc.dram_tensor("cmask", [128, 128], f32r, kind="ExternalInput").ap()
    y_d = nc.dram_tensor("y_part", [T, C], f32, kind="ExternalOutput").ap()
    import os
    taps = os.environ.get("KERNEL_TAPS") == "1"
    if taps:
        qt_o = nc.dram_tensor("qt_tap", [128, 2, T], f32r, kind="ExternalOutput").ap()
        kt_o = nc.dram_tensor("kt_tap", [128, 2, T], f32r, kind="ExternalOutput").ap()
        va_o = nc.dram_tensor("va_tap", [128, NKB, HG, 128], f32r, kind="ExternalOutput").ap()
        on_o = nc.dram_tensor("on_tap", [128, 2, T], f32r, kind="ExternalOutput").ap()

    with tile.TileContext(nc) as tc:
        with tc.tile_pool(name="persist", bufs=1) as pp, \
             tc.tile_pool(name="ps_s", bufs=2, space="PSUM") as ps_s, \
             tc.tile_pool(name="ps_acc", bufs=4, space="PSUM") as ps_acc:
            # ---- persistent SBUF ----
            wq = pp.tile([128, CC, F], f32r, tag="wq")
            wk = pp.tile([128, CC, F], f32r, tag="wk")
            wv = pp.tile([128, CC, F], f32r, tag="wv")
            wp = pp.tile([128, 2, C], f32r, tag="wp")
            bq = pp.tile([128, 2], f32, tag="bq")
            bk = pp.tile([128, 2], f32, tag="bk")
            cm = pp.tile([128, 128], f32r, tag="cm")
            qt = pp.tile([128, 2, T], f32r, tag="qt")   # q features 0..255 on (part, fb)
            kt = pp.tile([128, 2, T], f32r, tag="kt")
            va = pp.tile([128, NKB, HG, 128], f32r, tag="va")  # [V | ones] per (kb, h)
            on = pp.tile([128, 2, T], f32r, tag="on")   # normalized O^T, proj lhsT

            nc.sync.dma_start(wq[:], wqT_d.rearrange("(cc p) f -> p cc f", p=128))
            nc.sync.dma_start(wk[:], wkT_d.rearrange("(cc p) f -> p cc f", p=128))
            nc.sync.dma_start(wv[:], wvT_d.rearrange("(cc p) f -> p cc f", p=128))
            nc.sync.dma_start(wp[:], wpT_d.rearrange("(ic p) o -> p ic o", p=128))
            nc.sync.dma_start(bq[:], bq_d)
            nc.sync.dma_start(bk[:], bk_d)
            nc.sync.dma_start(cm[:], cm_d)
            nc.vector.memset(va[:, :, :, D:128].bitcast(mybir.dt.uint16), 16256)

            # ---- phase 1: QKV projections (x^T resident only here) ----
            with tc.tile_pool(name="xtp", bufs=1) as xtp:
                xt = xtp.tile([128, CC, T], f32r, tag="xt")
                for cc in range(CC):
                    nc.sync.dma_start(xt[:, cc, :], xT_d[128 * cc:128 * (cc + 1), :])

                # V first so attention can start as early as possible
                for tb in range(NTB):
                    v_ps = ps_s.tile([128, F], mybir.dt.float32, tag="s",
                                     name=f"v_ps{tb}")
                    for cc in range(CC):
                        nc.tensor.matmul(v_ps[:], xt[:, cc, 128 * tb:128 * (tb + 1)],
                                         wv[:, cc, :], start=(cc == 0),
                                         stop=(cc == CC - 1))
                    nc.scalar.copy(va[:, tb, :, 0:D],
                                   v_ps[:].rearrange("p (h d) -> p h d", h=HG))

                for fb in range(2):   # feature blocks (head pairs)
                    for name, w_t, b_t, dst in (("k", wk, bk, kt), ("q", wq, bq, qt)):
                        for tsp in range(2):   # pairs of 512-token spans
                            g_ps = [ps_s.tile([128, 512], mybir.dt.float32, tag="s",
                                              name=f"{name}_ps{fb}_{2*tsp+j}")
                                    for j in range(2)]
                            for cc in range(CC):
                                for j in range(2):
                                    ts = 2 * tsp + j
                                    nc.tensor.matmul(
                                        g_ps[j][:],
                                        w_t[:, cc, 128 * fb:128 * (fb + 1)],
                                        xt[:, cc, 512 * ts:512 * (ts + 1)],
                                        start=(cc == 0), stop=(cc == CC - 1))
                            for j in range(2):
                                ts = 2 * tsp + j
                                nc.scalar.activation(dst[:, fb, 512 * ts:512 * (ts + 1)],
                                                     g_ps[j][:], AF.Identity,
                                                     bias=b_t[:, fb:fb + 1], scale=1.0)

            # ---- phase 2: attention, head by head ----
            with tc.tile_pool(name="attn", bufs=1) as ap, \
                 tc.tile_pool(name="ptp", bufs=3) as ptp, \
                 tc.tile_pool(name="recp", bufs=2) as recp:
                for h in range(HG):
                    fb, po = h // 2, 64 * (h % 2)
                    oacc = [ps_acc.tile([128, 512], mybir.dt.float32, tag="acc",
                                        name=f"oacc{h}_{qi}") for qi in range(NQS)]
                    kb_last = [min(NKB - 1, 4 * qi + 3) for qi in range(NQS)]
                    for kb in range(NKB):
                        qlo = 128 * kb
                        pts = []
                        for ch in range(qlo, T, 1024):
                            clen = min(1024, T - ch)
                            s_ps = ps_s.tile([128, clen], mybir.dt.float32, tag="s",
                                             name=f"s{h}_{kb}_{ch}")
                            for ns in range(0, clen, 512):
                                n = min(512, clen - ns)
                                nc.tensor.matmul(
                                    s_ps[:, ns:ns + n],
                                    kt[po:po + 64, fb, qlo:qlo + 128],
                                    qt[po:po + 64, fb, ch + ns:ch + ns + n],
                                    start=True, stop=True)
                            pt = ptp.tile([128, clen], f32r, tag="pt",
                                          name=f"pt{h}_{kb}_{ch}")
                            nc.scalar.activation(pt[:], s_ps[:], AF.Exp,
                                                 bias=0.0, scale=SCALE)
                            if ch == qlo:  # causal mask on the diagonal block
                                nc.vector.tensor_mul(pt[:, 0:128], pt[:, 0:128], cm[:])
                            pts.append((ch, clen, pt))
                        for ch, clen, pt in pts:
                            qi0, qi1 = ch // 512, (ch + clen + 511) // 512
                            for qi in range(qi0, qi1):
                                glo, ghi = max(ch, 512 * qi), min(ch + clen, 512 * (qi + 1))
                                nc.tensor.matmul(
                                    oacc[qi][:, glo - 512 * qi:ghi - 512 * qi],
                                    va[:, kb, h, :],
                                    pt[:, glo - ch:ghi - ch],
                                    start=(kb == 0), stop=(kb == kb_last[qi]))
                    lns = recp.tile([64, T], mybir.dt.float32, tag="lns",
                                    name=f"lns{h}")
                    for qi in range(NQS):
                        nc.scalar.activation(lns[:, 512 * qi:512 * (qi + 1)],
                                             oacc[qi][64:128, :], AF.Ln)
                    rec = recp.tile([64, T], mybir.dt.float32, tag="rec",
                                    name=f"rec{h}")
                    nc.scalar.activation(rec[:], lns[:], AF.Exp, bias=0.0, scale=-1.0)
                    for qi in range(NQS):
                        nc.vector.tensor_mul(on[po:po + 64, fb, 512 * qi:512 * (qi + 1)],
                                             oacc[qi][0:64, :], rec[:, 512 * qi:512 * (qi + 1)])

                if taps:
                    nc.sync.dma_start(qt_o, qt[:])
                    nc.sync.dma_start(kt_o, kt[:])
                    nc.sync.dma_start(va_o, va[:])
                    nc.sync.dma_start(on_o, on[:])

                # ---- phase 3: output projection partial ----
                with tc.tile_pool(name="outp", bufs=3) as outp:
                    for tb in range(NTB):
                        osb = outp.tile([128, C], mybir.dt.float32, tag="osb",
                                        name=f"osb{tb}")
                        p_ps = [ps_s.tile([128, 512], mybir.dt.float32, tag="s",
                                          name=f"p_ps{tb}_{j}") for j in range(2)]
                        for ic in range(2):
                            for os_ in range(2):
                                nc.tensor.matmul(p_ps[os_][:],
                                                 on[:, ic, 128 * tb:128 * (tb + 1)],
                                                 wp[:, ic, 512 * os_:512 * (os_ + 1)],
                                                 start=(ic == 0), stop=(ic == 1))
                        for os_ in range(2):
                            nc.vector.tensor_copy(osb[:, 512 * os_:512 * (os_ + 1)],
                                                  p_ps[os_][:])
                        nc.sync.dma_start(y_d[128 * tb:128 * (tb + 1), :], osb[:])
    nc.compile()
    _NC_CACHE["nc"] = nc
    return nc


def _in_maps(x, w_attn, b_attn, w_proj):
    import ml_dtypes
    bf16 = ml_dtypes.bfloat16
    cmask = (np.arange(128)[None, :] >= np.arange(128)[:, None]).astype(bf16)
    maps = []
    for c in range(8):
        b, g = c // 4, c % 4
        r0 = F * g
        maps.append({
            "xT": np.ascontiguousarray(x[b].T).astype(bf16),
            "wqT": np.ascontiguousarray(w_attn[r0:r0 + F, :].T).astype(bf16),
            "wkT": np.ascontiguousarray(w_attn[C + r0:C + r0 + F, :].T).astype(bf16),
            "wvT": np.ascontiguousarray(w_attn[2 * C + r0:2 * C + r0 + F, :].T).astype(bf16),
            "wpT": np.ascontiguousarray(w_proj[:, r0:r0 + F].T).astype(bf16),
            "bq2": np.ascontiguousarray(b_attn[r0:r0 + F].reshape(2, 128).T),
            "bk2": np.ascontiguousarray(b_attn[C + r0:C + r0 + F].reshape(2, 128).T),
            "cmask": cmask,
        })
    return maps


def kernel(x, mask, W_attn, b_attn, W_proj, b_proj, _trace=False, _trace_kwargs=None):
    x = np.asarray(x, dtype=np.float32)
    W_attn = np.asarray(W_attn, dtype=np.float32)
    b_attn = np.asarray(b_attn, dtype=np.float32)
    W_proj = np.asarray(W_proj, dtype=np.float32)
    b_proj = np.asarray(b_proj, dtype=np.float32)

    nc = build_program()
    maps = _in_maps(x, W_attn, b_attn, W_proj)
    res = run_bass_kernel_spmd(nc, maps, core_ids=list(range(8)),
                               trace=_trace, **(_trace_kwargs or {}))

    b_eff = b_proj + W_proj @ b_attn[2 * C:3 * C]
    y = np.empty((B, T, C), dtype=np.float32)
    for b in range(B):
        acc = np.zeros((T, C), dtype=np.float32)
        for g in range(4):
            acc += res.results[4 * b + g]["y_part"]
        y[b] = acc + b_eff
    kernel.last_results = res
    return y


# revision 6
# speedup vs baseline: 1.1234x; 1.0518x over previous
"""Causal self-attention (B=2, T=2048, C=1024, H=16) on 8 TRN2 NeuronCores.

Sharding: core c = (batch b=c//4, head-group g=c%4); each core owns 4 heads
of one batch element. Per core:
  Q^T/K^T = W_slice @ x^T (+bias)          [feature-major layouts]
  V       = x @ Wv_slice^T                 [token-major, augmented with ones]
  S^T     = K^T_blk.T @ Q^T  (keys on partitions, queries on free dim)
  P^T     = exp(S^T / 32) * causal_mask
  [O^T; s]= [V | 1].T @ P^T  (PSUM-accumulated over key blocks)
  O_n^T   = O^T * 1/s
  y_part  = O_n^T.T @ Wp_slice^T           [proj partial over 256 of 1024 dims]
Host: y[b] = sum_g y_part[b,g] + (b_proj + W_proj @ b_attn[2C:3C]).
All matmuls run in float32r (~2e-4 rel err, 4x faster than fp32).
"""
import sys

sys.path.insert(0, "/opt/trn_rl_repo")

import numpy as np

import concourse.bass as bass  # noqa: F401
import concourse.mybir as mybir
import concourse.tile as tile
from concourse import bacc
from concourse.bass_utils import run_bass_kernel_spmd

B, T, C, H = 2, 2048, 1024, 16
HG = 4            # heads per core
D = C // H        # 64
F = HG * D        # 256 features per core for each of q/k/v
CC = C // 128     # 8 contraction chunks
NKB = T // 128    # 16 key blocks
NTB = T // 128    # 16 token blocks
NQS = T // 512    # 4 query spans
SCALE = 1.0 / 32.0
AF = mybir.ActivationFunctionType

_NC_CACHE = {}


def build_program():
    if "nc" in _NC_CACHE:
        return _NC_CACHE["nc"]
    f32r = mybir.dt.bfloat16  # matmul operand dtype (fp32 PSUM accum)
    f32 = mybir.dt.float32
    nc = bacc.Bacc("TRN2", target_bir_lowering=False, debug=False, num_devices=8)

    xT_d = nc.dram_tensor("xT", [C, T], f32r, kind="ExternalInput").ap()
    wqT_d = nc.dram_tensor("wqT", [C, F], f32r, kind="ExternalInput").ap()
    wkT_d = nc.dram_tensor("wkT", [C, F], f32r, kind="ExternalInput").ap()
    wvT_d = nc.dram_tensor("wvT", [C, F], f32r, kind="ExternalInput").ap()
    wpT_d = nc.dram_tensor("wpT", [F, C], f32r, kind="ExternalInput").ap()
    bq_d = nc.dram_tensor("bq2", [128, 2], f32, kind="ExternalInput").ap()
    bk_d = nc.dram_tensor("bk2", [128, 2], f32, kind="ExternalInput").ap()
    cm_d = nc.dram_tensor("cmask", [128, 128], f32r, kind="ExternalInput").ap()
    y_d = nc.dram_tensor("y_part", [T, C], f32, kind="ExternalOutput").ap()
    import os
    taps = os.environ.get("KERNEL_TAPS") == "1"
    if taps:
        qt_o = nc.dram_tensor("qt_tap", [128, 2, T], f32r, kind="ExternalOutput").ap()
        kt_o = nc.dram_tensor("kt_tap", [128, 2, T], f32r, kind="ExternalOutput").ap()
        va_o = nc.dram_tensor("va_tap", [128, NKB, HG, 128], f32r, kind="ExternalOutput").ap()
        on_o = nc.dram_tensor("on_tap", [128, 2, T], f32r, kind="ExternalOutput").ap()

    with tile.TileContext(nc) as tc:
        with tc.tile_pool(name="persist", bufs=1) as pp, \
             tc.tile_pool(name="ps_s", bufs=2, space="PSUM") as ps_s, \
             tc.tile_pool(name="ps_acc", bufs=4, space="PSUM") as ps_acc:
            # ---- persistent SBUF ----
            wq = pp.tile([128, CC, F], f32r, tag="wq")
            wk = pp.tile([128, CC, F], f32r, tag="wk")
            wv = pp.tile([128, CC, F], f32r, tag="wv")
            wp = pp.tile([128, 2, C], f32r, tag="wp")
            bq = pp.tile([128, 2], f32, tag="bq")
            bk = pp.tile([128, 2], f32, tag="bk")
            cm = pp.tile([128, 128], f32r, tag="cm")
            qt = pp.tile([128, 2, T], f32r, tag="qt")   # q features 0..255 on (part, fb)
            kt = pp.tile([128, 2, T], f32r, tag="kt")
            va = pp.tile([128, NKB, HG, 128], f32r, tag="va")  # [V | ones] per (kb, h)
            on = pp.tile([128, 2, T], f32r, tag="on")   # normalized O^T, proj lhsT

            nc.sync.dma_start(wq[:], wqT_d.rearrange("(cc p) f -> p cc f", p=128))
            nc.sync.dma_start(wk[:], wkT_d.rearrange("(cc p) f -> p cc f", p=128))
            nc.sync.dma_start(wv[:], wvT_d.rearrange("(cc p) f -> p cc f", p=128))
            nc.sync.dma_start(wp[:], wpT_d.rearrange("(ic p) o -> p ic o", p=128))
            nc.sync.dma_start(bq[:], bq_d)
            nc.sync.dma_start(bk[:], bk_d)
            nc.sync.dma_start(cm[:], cm_d)
            nc.vector.memset(va[:, :, :, D:128].bitcast(mybir.dt.uint16), 16256)

            # ---- phase 1: QKV projections (x^T resident only here) ----
            with tc.tile_pool(name="xtp", bufs=1) as xtp:
                xt = xtp.tile([128, CC, T], f32r, tag="xt")
                for cc in range(CC):
                    nc.sync.dma_start(xt[:, cc, :], xT_d[128 * cc:128 * (cc + 1), :])

                # V first so attention can start as early as possible
                for tb in range(NTB):
                    v_ps = ps_s.tile([128, F], mybir.dt.float32, tag="s",
                                     name=f"v_ps{tb}")
                    for cc in range(CC):
                        nc.tensor.matmul(v_ps[:], xt[:, cc, 128 * tb:128 * (tb + 1)],
                                         wv[:, cc, :], start=(cc == 0),
                                         stop=(cc == CC - 1))
                    nc.scalar.copy(va[:, tb, :, 0:D],
                                   v_ps[:].rearrange("p (h d) -> p h d", h=HG))

                for fb in range(2):   # feature blocks (head pairs)
                    for name, w_t, b_t, dst in (("k", wk, bk, kt), ("q", wq, bq, qt)):
                        for ts in range(NQS):
                            g_ps = ps_s.tile([128, 512], mybir.dt.float32, tag="s",
                                             name=f"{name}_ps{fb}_{ts}")
                            for cc in range(CC):
                                nc.tensor.matmul(
                                    g_ps[:],
                                    w_t[:, cc, 128 * fb:128 * (fb + 1)],
                                    xt[:, cc, 512 * ts:512 * (ts + 1)],
                                    start=(cc == 0), stop=(cc == CC - 1))
                            nc.scalar.activation(dst[:, fb, 512 * ts:512 * (ts + 1)],
                                                 g_ps[:], AF.Identity,
                                                 bias=b_t[:, fb:fb + 1], scale=1.0)

            # ---- phase 2: attention, head by head ----
            with tc.tile_pool(name="attn", bufs=1) as ap, \
                 tc.tile_pool(name="ptp", bufs=3) as ptp, \
                 tc.tile_pool(name="recp", bufs=2) as recp:
                for h in range(HG):
                    fb, po = h // 2, 64 * (h % 2)
                    oacc = [ps_acc.tile([128, 512], mybir.dt.float32, tag="acc",
                                        name=f"oacc{h}_{qi}") for qi in range(NQS)]
                    kb_last = [min(NKB - 1, 4 * qi + 3) for qi in range(NQS)]
                    for kb in range(NKB):
                        qlo = 128 * kb
                        for ch in range(qlo, T, 1024):
                            clen = min(1024, T - ch)
                            s_ps = ps_s.tile([128, clen], mybir.dt.float32, tag="s",
                                             name=f"s{h}_{kb}_{ch}")
                            for ns in range(0, clen, 512):
                                n = min(512, clen - ns)
                                nc.tensor.matmul(
                                    s_ps[:, ns:ns + n],
                                    kt[po:po + 64, fb, qlo:qlo + 128],
                                    qt[po:po + 64, fb, ch + ns:ch + ns + n],
                                    start=True, stop=True)
                            pt = ptp.tile([128, clen], f32r, tag="pt",
                                          name=f"pt{h}_{kb}_{ch}")
                            nc.scalar.activation(pt[:], s_ps[:], AF.Exp,
                                                 bias=0.0, scale=SCALE)
                            if ch == qlo:  # causal mask on the diagonal block
                                nc.vector.tensor_mul(pt[:, 0:128], pt[:, 0:128], cm[:])
                            qi0, qi1 = ch // 512, (ch + clen + 511) // 512
                            for qi in range(qi0, qi1):
                                glo, ghi = max(ch, 512 * qi), min(ch + clen, 512 * (qi + 1))
                                nc.tensor.matmul(
                                    oacc[qi][:, glo - 512 * qi:ghi - 512 * qi],
                                    va[:, kb, h, :],
                                    pt[:, glo - ch:ghi - ch],
                                    start=(kb == 0), stop=(kb == kb_last[qi]))
                    for qi in range(NQS):
                        rec = recp.tile([64, 512], mybir.dt.float32, tag="rec",
                                        name=f"rec{h}_{qi}")
                        nc.vector.reciprocal(rec[:], oacc[qi][64:128, :])
                        nc.vector.tensor_mul(on[po:po + 64, fb, 512 * qi:512 * (qi + 1)],
                                             oacc[qi][0:64, :], rec[:])

                if taps:
                    nc.sync.dma_start(qt_o, qt[:])
                    nc.sync.dma_start(kt_o, kt[:])
                    nc.sync.dma_start(va_o, va[:])
                    nc.sync.dma_start(on_o, on[:])

                # ---- phase 3: output projection partial ----
                with tc.tile_pool(name="outp", bufs=3) as outp:
                    for tb in range(NTB):
                        osb = outp.tile([128, C], mybir.dt.float32, tag="osb",
                                        name=f"osb{tb}")
                        for os_ in range(2):
                            p_ps = ps_s.tile([128, 512], mybir.dt.float32, tag="s",
                                             name=f"p_ps{tb}_{os_}")
                            for ic in range(2):
                                nc.tensor.matmul(p_ps[:],
                                                 on[:, ic, 128 * tb:128 * (tb + 1)],
                                                 wp[:, ic, 512 * os_:512 * (os_ + 1)],
                                                 start=(ic == 0), stop=(ic == 1))
                            nc.vector.tensor_copy(osb[:, 512 * os_:512 * (os_ + 1)],
                                                  p_ps[:])
                        nc.sync.dma_start(y_d[128 * tb:128 * (tb + 1), :], osb[:])
    nc.compile()
    _NC_CACHE["nc"] = nc
    return nc


def _in_maps(x, w_attn, b_attn, w_proj):
    import ml_dtypes
    bf16 = ml_dtypes.bfloat16
    cmask = (np.arange(128)[None, :] >= np.arange(128)[:, None]).astype(bf16)
    maps = []
    for c in range(8):
        b, g = c // 4, c % 4
        r0 = F * g
        maps.append({
            "xT": np.ascontiguousarray(x[b].T).astype(bf16),
            "wqT": np.ascontiguousarray(w_attn[r0:r0 + F, :].T).astype(bf16),
            "wkT": np.ascontiguousarray(w_attn[C + r0:C + r0 + F, :].T).astype(bf16),
            "wvT": np.ascontiguousarray(w_attn[2 * C + r0:2 * C + r0 + F, :].T).astype(bf16),
            "wpT": np.ascontiguousarray(w_proj[:, r0:r0 + F].T).astype(bf16),
            "bq2": np.ascontiguousarray(b_attn[r0:r0 + F].reshape(2, 128).T),
            "bk2": np.ascontiguousarray(b_attn[C + r0:C + r0 + F].reshape(2, 128).T),
            "cmask": cmask,
        })
    return maps


def kernel(x, mask, W_attn, b_attn, W_proj, b_proj, _trace=False, _trace_kwargs=None):
    x = np.asarray(x, dtype=np.float32)
    W_attn = np.asarray(W_attn, dtype=np.float32)
    b_attn = np.asarray(b_attn, dtype=np.float32)
    W_proj = np.asarray(W_proj, dtype=np.float32)
    b_proj = np.asarray(b_proj, dtype=np.float32)

    nc = build_program()
    maps = _in_maps(x, W_attn, b_attn, W_proj)
    res = run_bass_kernel_spmd(nc, maps, core_ids=list(range(8)),
                               trace=_trace, **(_trace_kwargs or {}))

    b_eff = b_proj + W_proj @ b_attn[2 * C:3 * C]
    y = np.empty((B, T, C), dtype=np.float32)
    for b in range(B):
        acc = np.zeros((T, C), dtype=np.float32)
        for g in range(4):
            acc += res.results[4 * b + g]["y_part"]
        y[b] = acc + b_eff
    kernel.last_results = res
    return y
